# revision 13
# baseline (speedup 1.0000x reference)
"""CenterPooling (CornerNet) Trainium2 kernel — 8 NeuronCores.

Sharding: 8 cores = 4 batches x 2 H-halves.  Each core gets a host-padded
input slab (3 halo rows each side, zero W-pad columns).

Key algebraic simplifications:
 - cummax(reverse) then cummax(forward) along an axis == global max along
   that axis, broadcast.  So the up branch only needs per-row maxes over W
   ([C, H]) and the down branch per-column maxes over H ([C, W]).
 - BN (eval mode) folds into conv weights/bias on the host; BN scale > 0 so
   max-reduction commutes with the affine+ReLU epilogue.
 - The merge conv's input is rank-structured: updown[c,h,w] = u[c,h] + d[c,w],
   so the 3x3 merge conv SEPARATES into tiny 1-D convs: an h-conv of u
   ([C, H] -> A(o,h), with 3 w-boundary classes of kx-summed weights) plus a
   w-conv of d ([C, W] -> B(o,w), with h-boundary corrections applied
   data-driven via per-row selector vectors).
 - Down-branch col-max needs a cross-half combine: pairwise AllReduce-max of
   a tiny [256, 128] tile.
 - H-pad semantics at the global top/bottom are handled data-driven (SPMD
   uniform program): a validity mask zeroes invalid u rows, and a per-row
   -1e30 bias on relu1 clamps out-of-range rows to the zero-pad value.

Precision: the up/down/c1 convs run as fp8e4 (TRN FP8_EXP4, max 240)
DoubleRow matmuls; the output conv block (c2) and the tiny separable merge
matmuls stay bf16 (fp8 there pushes rel_err past the 2e-2 gate; measured
~1e-2 with this split).  Accumulation is always fp32 PSUM.  Output is
stored bf16 and upcast on the host (~0.1% extra L2, halves store traffic).

Schedule notes (v2):
 - Input DMA descriptors are issued from BOTH hardware DGE queues (sync +
   scalar) so descriptor generation (~0.7us each) does not serialize the
   startup: weights on sync, the x8 slab on scalar.
 - ufin/umask are produced incrementally per up-conv block, so the A-conv
   matmuls can start the moment the last up block drains; the B-conv MMs
   run right after A, keeping the PE busy while the A/B epilogue chains
   (psum copies + afull/afdl/afdr) run on scalar/vector.
 - relu1 assembly is batched per 4-row block with broadcast APs (one STT +
   one add + one ACT per block-cot instead of per-row ops), shortening the
   exposed chain at the conv tail.
 - the last c2 block is split 2+2 rows to shorten the final ACT+store tail.
"""

import sys

sys.path.insert(0, "/opt/trn_rl_repo")

import numpy as np
import ml_dtypes

import concourse.bacc as bacc
import concourse.tile as tile
import concourse.bass as bass
from concourse import mybir, bass_utils

BF16 = mybir.dt.bfloat16
FP8 = mybir.dt.float8e4
F32 = mybir.dt.float32
NP_BF16 = ml_dtypes.bfloat16
NP_FP8 = ml_dtypes.float8_e4m3  # IEEE e4m3, max 240 == TRN FP8_EXP4

N_CORES = 8
B, CIN, C, H, W = 4, 256, 256, 128, 128
G = 3            # halo rows on each side of the 64 owned rows
HS = 64 + 2 * G  # 70 slab rows
WP = W + 2       # 130 (zero-pad col on each side) — bf16 r1 slab
WPX = 144        # fp8 x slab width: 16B-aligned row and plane strides
EPS = 1e-5
NEG = -1e30
QCLIP = 224.0    # fp8 absmax target (headroom under the 240 max)

RELU = mybir.ActivationFunctionType.Relu
AX_X = mybir.AxisListType.X
ALU = mybir.AluOpType
DROW = mybir.MatmulPerfMode.DoubleRow


def _mm_group(nc, ps_ap, mms, perf_mode=None):
    n = len(mms)
    for k, (lhsT, rhs) in enumerate(mms):
        nc.tensor.matmul(ps_ap, lhsT, rhs, start=(k == 0), stop=(k == n - 1),
                         perf_mode=perf_mode)


def _conv3_mms8(wtile, x8, s, nr, cot):
    """The 9 (ky,kx) DoubleRow matmuls of a 3x3 conv: output rows s..s+nr-1.

    wtile is [128ci, 2cit, 2cot, 9j, 128co]; the lhsT AP [128, 2, 128] takes
    the cit planes at stride 2*9*128 (16B-aligned as DoubleRow requires)."""
    mms = []
    for ky in range(3):
        for kx in range(3):
            mms.append((wtile[:, :, cot, ky * 3 + kx, :],
                        x8[:, :, s + ky - 1:s + ky - 1 + nr, kx:kx + W]))
    return mms


def _conv3_mms(wtile, src, s, nr, cot):
    """The 18 (ci,ky,kx) bf16 matmuls of a 3x3 conv block."""
    mms = []
    for cit in range(2):
        for ky in range(3):
            for kx in range(3):
                j = ((ky * 3 + kx) * 2 + cit) * 2 + cot
                mms.append((wtile[:, j, :], src[cit][:, s + ky - 1:s + ky - 1 + nr, kx:kx + W]))
    return mms


def _build(qs):
    nc = bacc.Bacc("TRN2", target_bir_lowering=False, debug=False,
                   num_devices=N_CORES)

    x8_d = nc.dram_tensor("x8", [128, 2, HS, WPX], FP8, kind="ExternalInput")
    # (cit, cot) are OUTERMOST in dram so each per-(cit,cot) DMA is fully
    # contiguous per partition; the first down MM then only gates on the
    # cot=0 halves.
    wup_d = nc.dram_tensor("wup", [2, 2, 128, 9, 128], FP8, kind="ExternalInput")
    wdn_d = nc.dram_tensor("wdn", [2, 2, 128, 9, 128], FP8, kind="ExternalInput")
    wc1_d = nc.dram_tensor("wc1", [2, 128, 2, 128], FP8, kind="ExternalInput")
    wc2_d = nc.dram_tensor("wc2", [128, 36, 128], BF16, kind="ExternalInput")
    wa_d = nc.dram_tensor("wa", [128, 36, 128], BF16, kind="ExternalInput")
    wb_d = nc.dram_tensor("wb", [128, 36, 128], BF16, kind="ExternalInput")
    bias_d = nc.dram_tensor("biases", [128, 8], F32, kind="ExternalInput")
    hv_d = nc.dram_tensor("hv", [128, HS], F32, kind="ExternalInput")
    pnegb_d = nc.dram_tensor("pnegb", [128, HS], F32, kind="ExternalInput")
    htop_d = nc.dram_tensor("htopneg", [128, HS], F32, kind="ExternalInput")
    hbot_d = nc.dram_tensor("hbotneg", [128, HS], F32, kind="ExternalInput")
    out_d = nc.dram_tensor("out", [2, 128, 64, W], BF16, kind="ExternalOutput")

    with tile.TileContext(nc) as tc:
        with tc.tile_pool(name="const", bufs=1) as constp, \
             tc.tile_pool(name="acts", bufs=1) as actp, \
             tc.tile_pool(name="psum", bufs=6, space="PSUM") as psp, \
             tc.tile_pool(name="ostage", bufs=6) as osp, \
             tc.tile_pool(name="dram", bufs=1, space="DRAM") as dramp:

            # --- input DMA: weights on the sync queue, x8 on the scalar
            # queue (both are hardware DGE engines) so descriptor
            # generation overlaps and the first conv can start sooner.
            wdn = constp.tile([128, 2, 2, 9, 128], FP8)
            for cot in range(2):
                for cit in range(2):
                    nc.sync.dma_start(wdn[:, cit, cot, :, :],
                                      wdn_d.ap()[cit, cot, :, :, :])

            x8 = actp.tile([128, 2, HS, WPX], FP8, name="x8")
            # first 7 rows of both cits land first (feeds the first down and
            # up blocks), then 16-row chunks
            row_chunks = [(0, 7), (7, 23), (23, 39), (39, 55), (55, HS)]
            for r0, r1_ in row_chunks:
                for cit in range(2):
                    nc.scalar.dma_start(x8[:, cit, r0:r1_, :], x8_d.ap()[:, cit, r0:r1_, :])

            wup = constp.tile([128, 2, 2, 9, 128], FP8)
            for cot in range(2):
                for cit in range(2):
                    nc.sync.dma_start(wup[:, cit, cot, :, :],
                                      wup_d.ap()[cit, cot, :, :, :])
            wc1 = constp.tile([128, 2, 2, 128], FP8)
            for cit in range(2):
                nc.sync.dma_start(wc1[:, cit, :, :], wc1_d.ap()[cit, :, :, :])
            biases = constp.tile([128, 8], F32)
            nc.sync.dma_start(biases[:, :], bias_d.ap())
            hv = constp.tile([128, HS], F32)
            nc.sync.dma_start(hv[:, :], hv_d.ap())
            wa = constp.tile([128, 36, 128], BF16)
            nc.sync.dma_start(wa[:, :, :], wa_d.ap())
            wb = constp.tile([128, 36, 128], BF16)
            nc.sync.dma_start(wb[:, :, :], wb_d.ap())
            pnegb = constp.tile([128, HS], F32)
            nc.sync.dma_start(pnegb[:, :], pnegb_d.ap())
            htopneg = constp.tile([128, HS], F32)
            nc.sync.dma_start(htopneg[:, :], htop_d.ap())
            hbotneg = constp.tile([128, HS], F32)
            nc.sync.dma_start(hbotneg[:, :], hbot_d.ap())
            wc2 = constp.tile([128, 36, 128], BF16)
            nc.sync.dma_start(wc2[:, :, :], wc2_d.ap())

            r1 = []
            for cit in range(2):
                t2 = actp.tile([128, HS, WP], BF16, name=f"r1{cit}")
                nc.vector.memset(t2[:, :, 0], 0.0)
                nc.vector.memset(t2[:, :, WP - 1], 0.0)
                r1.append(t2)

            uraw, ufin, umask, dacc, dmax, dfin = [], [], [], [], [], []
            for cot in range(2):
                uraw.append(actp.tile([128, HS], F32, name=f"uraw{cot}"))
                ufin.append(actp.tile([128, HS], F32, name=f"ufin{cot}"))
                umask.append(actp.tile([128, HS], BF16, name=f"umask{cot}"))
                t = actp.tile([128, W], F32, name=f"dacc{cot}")
                nc.vector.memset(t[:, :], -3e38)
                dacc.append(t)
                dmax.append(actp.tile([128, W], F32, name=f"dmax{cot}"))
                dfin.append(actp.tile([128, W], F32, name=f"dfin{cot}"))

            # ---- down branch: fp8 conv over the 64 owned rows, col-max over H ----
            for i in range(16):
                s = G + 4 * i
                for cot in range(2):
                    ps = psp.tile([128, 4, 128], F32, tag="ps", name="ps_dn", bufs=5)
                    _mm_group(nc, ps[:, :, :], _conv3_mms8(wdn, x8, s, 4, cot),
                              perf_mode=DROW)
                    for rr in range(4):
                        nc.vector.tensor_max(dacc[cot][:, :], dacc[cot][:, :], ps[:, rr, :])

            # pairwise (same-batch) AllReduce-max to get the global col-max
            # (values carry the sx*swdn quant scale; both group members match)
            cc_in = dramp.tile([256, W], F32)
            cc_out = dramp.tile([256, W], F32)
            for cot in range(2):
                nc.sync.dma_start(cc_in[cot * 128:(cot + 1) * 128, :], dacc[cot][:, :])
            nc.gpsimd.collective_compute(
                "AllReduce", ALU.max,
                replica_groups=[[0, 1], [2, 3], [4, 5], [6, 7]],
                ins=[cc_in.opt()], outs=[cc_out.opt()])
            for cot in range(2):
                nc.sync.dma_start(dmax[cot][:, :], cc_out[cot * 128:(cot + 1) * 128, :])

            # ---- up branch: fp8 conv over rows [1, 69), row-max over W ----
            # ufin/umask are produced per block; the merge pieces (dfin/dpad,
            # the B conv, the first half of the A conv) and the first relu1
            # blocks are woven INTO the up-conv stream so no dependency chain
            # is exposed when the up conv drains.
            NA = 64 + 2        # A-conv output rows 2..67
            NA1 = 40           # half 1: rows 2..41  (needs umask 1..42)
            NA2 = NA - NA1     # half 2: rows 42..67 (needs umask 41..68)
            dpad = [None, None]
            asb = [[None, None, None], [None, None, None]]
            bt = [[None, None, None], [None, None, None]]
            afull, afdl, afdr = [], [], []
            for cot in range(2):
                afull.append(actp.tile([128, HS], F32, name=f"afull{cot}"))
                afdl.append(actp.tile([128, HS], F32, name=f"afdl{cot}"))
                afdr.append(actp.tile([128, HS], F32, name=f"afdr{cot}"))

            def emit_dfin_dpad():
                for cot in range(2):
                    nc.scalar.activation(dfin[cot][:, :], dmax[cot][:, :], RELU,
                                         bias=biases[:, 2 + cot:3 + cot],
                                         scale=qs["dn"][cot])
                    t = actp.tile([128, WP], BF16, name=f"dpad{cot}")
                    nc.vector.memset(t[:, :], 0.0)
                    nc.vector.tensor_copy(t[:, 1:W + 1], dfin[cot][:, :])
                    dpad[cot] = t

            def emit_a_half(r0, na):
                # A_cls(o,h): 1-D h-conv of umask with kx-summed merge
                # weights; cls 0=M (interior w), 1=L (w=0), 2=R (w=127).
                # Output rows r0..r0+na-1 into asb columns [r0-2, r0-2+na).
                for cls in range(3):
                    for cot in range(2):
                        psa_t = psp.tile([128, 4, 128], F32, tag="ps2", name="ps_a", bufs=3)
                        mms = []
                        for cit in range(2):
                            for ky in range(3):
                                j = ((cls * 3 + ky) * 2 + cit) * 2 + cot
                                mms.append((wa[:, j, :],
                                            umask[cit][:, r0 - 1 + ky:r0 - 1 + ky + na]))
                        _mm_group(nc, psa_t[:, 0, 0:na], mms)
                        if asb[cot][cls] is None:
                            asb[cot][cls] = actp.tile([128, NA], F32, name=f"asb{cls}{cot}")
                        nc.scalar.copy(asb[cot][cls][:, r0 - 2:r0 - 2 + na],
                                       psa_t[:, 0, 0:na])
                # afull = A_M + bias_pc1 + pneg (per relu1 row);
                # afdL/afdR = A_L - A_M / A_R - A_M (w-edge fixups, pre-ReLU).
                for cot in range(2):
                    nc.vector.scalar_tensor_tensor(
                        afull[cot][:, r0:r0 + na], asb[cot][0][:, r0 - 2:r0 - 2 + na],
                        biases[:, 4 + cot:5 + cot], pnegb[:, r0:r0 + na],
                        op0=ALU.add, op1=ALU.add)
                    nc.vector.tensor_sub(afdl[cot][:, r0:r0 + na],
                                         asb[cot][1][:, r0 - 2:r0 - 2 + na],
                                         asb[cot][0][:, r0 - 2:r0 - 2 + na])
                    nc.vector.tensor_sub(afdr[cot][:, r0:r0 + na],
                                         asb[cot][2][:, r0 - 2:r0 - 2 + na],
                                         asb[cot][0][:, r0 - 2:r0 - 2 + na])

            def emit_b():
                # B_var(o,w): 1-D w-conv of dpad with ky-summed merge
                # weights; var 0=M (all ky), 1=ky0 only, 2=ky2 only.
                for var in range(3):
                    for cot in range(2):
                        psb_t = psp.tile([128, 4, 128], F32, tag="ps2", name="ps_b", bufs=3)
                        mms = []
                        for cit in range(2):
                            for kx in range(3):
                                j = ((var * 3 + kx) * 2 + cit) * 2 + cot
                                mms.append((wb[:, j, :], dpad[cit][:, kx:kx + W]))
                        _mm_group(nc, psb_t[:, 0, :], mms)
                        t = actp.tile([128, 128], F32, name=f"bt{var}{cot}")
                        nc.vector.tensor_copy(t[:, :], psb_t[:, 0, :])
                        bt[cot][var] = t

            def btaf_prep(s, nr, cot):
                # btaf[:, r, w] = bt_M[:, w] + afull[:, s+r] + edge fixups —
                # precomputed on vector OFF the c1->relu1 critical chain.
                t = osp.tile([128, 4, 128], F32, tag="btaf", name="btaf", bufs=6)
                bt_b = bt[cot][0][:, :].unsqueeze(1).broadcast_to([128, nr, 128])
                af_b = afull[cot][:, s:s + nr].unsqueeze(2).broadcast_to([128, nr, 128])
                nc.vector.tensor_add(t[:, 0:nr, :], bt_b, af_b)
                nc.vector.tensor_add(t[:, 0:nr, 0], t[:, 0:nr, 0], afdl[cot][:, s:s + nr])
                nc.vector.tensor_add(t[:, 0:nr, W - 1], t[:, 0:nr, W - 1], afdr[cot][:, s:s + nr])
                for r in range(nr):
                    sr = s + r
                    if sr == G:
                        nc.vector.scalar_tensor_tensor(
                            t[:, r, :], bt[cot][1][:, :], htopneg[:, sr:sr + 1],
                            t[:, r, :], op0=ALU.mult, op1=ALU.add)
                    if sr == HS - G - 1:
                        nc.vector.scalar_tensor_tensor(
                            t[:, r, :], bt[cot][2][:, :], hbotneg[:, sr:sr + 1],
                            t[:, r, :], op0=ALU.mult, op1=ALU.add)
                return t

            btafs = {}

            def emit_relu1_prep(s, nr):
                # emitted one step AHEAD of the mm stage so the vector queue
                # has the btaf ready when the c1 MM issues
                btafs[s] = [btaf_prep(s, nr, cot) for cot in range(2)]

            def emit_relu1_mms(s, nr):
                # relu1 = relu(c1(x) + A + B + bias): c1 MM -> STT -> ACT
                for cot in range(2):
                    btaf = btafs[s][cot]
                    ps = psp.tile([128, 4, 128], F32, tag="ps", name="ps_p", bufs=5)
                    nc.tensor.matmul(ps[:, 0:nr, :], wc1[:, :, cot, :],
                                     x8[:, :, s:s + nr, 1:W + 1],
                                     start=True, stop=True, perf_mode=DROW)
                    nc.vector.scalar_tensor_tensor(
                        ps[:, 0:nr, :], ps[:, 0:nr, :], qs["c1"][cot],
                        btaf[:, 0:nr, :], op0=ALU.mult, op1=ALU.add)
                    nc.scalar.activation(r1[cot][:, s:s + nr, 1:W + 1], ps[:, 0:nr, :],
                                         RELU, bias=0.0, scale=1.0)

            # up-conv blocks b=1..17 with merge work woven in
            for b in range(1, 18):
                s = 1 + 4 * (b - 1)
                for cot in range(2):
                    ps = psp.tile([128, 4, 128], F32, tag="ps", name="ps_up", bufs=5)
                    _mm_group(nc, ps[:, :, :], _conv3_mms8(wup, x8, s, 4, cot),
                              perf_mode=DROW)
                    nc.vector.reduce_max(uraw[cot][:, s:s + 4], ps[:, :, :], axis=AX_X)
                    nc.scalar.activation(ufin[cot][:, s:s + 4], uraw[cot][:, s:s + 4],
                                         RELU, bias=biases[:, cot:cot + 1],
                                         scale=qs["up"][cot])
                    nc.vector.tensor_mul(umask[cot][:, s:s + 4], ufin[cot][:, s:s + 4],
                                         hv[:, s:s + 4])
                if b == 10:
                    emit_dfin_dpad()       # collective done long before
                elif b == 11:
                    emit_a_half(2, NA1)    # umask rows 1..42 available
                    emit_b()
                elif b in (12, 14, 16):
                    # a few early relu1 blocks, spread so the vector queue
                    # keeps pace with the up conv; preps go one up-block
                    # ahead of the matmul stage
                    emit_relu1_prep(2 + 4 * ((b - 12) // 2), 4)
                elif b in (13, 15, 17):
                    emit_relu1_mms(2 + 4 * ((b - 13) // 2), 4)
                    if b == 17:
                        emit_relu1_prep(14, 4)

            # ---- A half 2, then relu1 and c2 blocks interleaved so the PE
            # always has a c2 block queued while relu1 chains run ----
            emit_a_half(2 + NA1, NA2)

            # ---- output conv block (bf16 direct), interleaved with the
            # remaining relu1 blocks ----
            def emit_c2_block(s, nr):
                for cot in range(2):
                    ps = psp.tile([128, 4, 128], F32, tag="ps2", name="ps_c2", bufs=3)
                    _mm_group(nc, ps[:, 0:nr, :], _conv3_mms(wc2, r1, s, nr, cot))
                    ot = osp.tile([128, 4, 128], BF16, name="ot")
                    nc.scalar.activation(ot[:, 0:nr, :], ps[:, 0:nr, :], RELU,
                                         bias=biases[:, 6 + cot:7 + cot], scale=1.0)
                    if s >= G + 56:
                        # split the tail stores across rings so the last
                        # store's serial latency is halved
                        for r in range(0, nr, 2):
                            nc.sync.dma_start(out_d.ap()[cot, :, s - G + r:s - G + r + 2, :],
                                              ot[:, r:r + 2, :])
                    else:
                        nc.sync.dma_start(out_d.ap()[cot, :, s - G:s - G + nr, :], ot[:, 0:nr, :])

            r1_rest = [(14 + 4 * i, 4) for i in range(13)] + [(66, 2)]
            c2blocks = [(G + 4 * i, 4) for i in range(15)] + [(G + 60, 2), (G + 62, 2)]
            for k in range(len(c2blocks)):
                if k < len(r1_rest):
                    emit_relu1_mms(*r1_rest[k])
                if k + 1 < len(r1_rest):
                    emit_relu1_prep(*r1_rest[k + 1])
                emit_c2_block(*c2blocks[k])

    nc.compile()
    return nc


def _pack3(w):
    # [256o, 256i, 3, 3] -> [128ci, j, 128co], j = ((ky*3+kx)*2+cit)*2+cot
    a = w.reshape(2, 128, 2, 128, 3, 3).transpose(3, 4, 5, 2, 0, 1)
    return np.ascontiguousarray(a.reshape(128, 36, 128)).astype(NP_BF16)


def _q8(a, s):
    return np.clip(a * s, -240.0, 240.0).astype(NP_FP8)


def _pack3_fp8(w, s_cot):
    # [256o, 256i, 3, 3] -> [2cit, 2cot, 128ci, j=ky*3+kx, 128co] fp8,
    # scaled per output-channel tile (cot)
    ws = w * np.repeat(s_cot, 128)[:, None, None, None]
    a = ws.reshape(2, 128, 2, 128, 3, 3).transpose(2, 0, 3, 4, 5, 1)
    # dims now [cit, cot, ci, ky, kx, co]
    return np.ascontiguousarray(_q8(a.reshape(2, 2, 128, 9, 128), 1.0))


def _pack1_fp8(w, s_cot):
    # [256o, 256i, 1, 1] -> [2cit, 128ci, cot, 128co] fp8
    ws = w[:, :, 0, 0] * np.repeat(s_cot, 128)[:, None]
    a = ws.reshape(2, 128, 2, 128).transpose(2, 3, 0, 1)
    return np.ascontiguousarray(_q8(a, 1.0))


def _pack_sep(wk3):
    # packs a [3var/cls, 3k, 256, 256] stack into [128ci, j, 128co],
    # j = ((v*3+k)*2+cit)*2+cot
    a = wk3.reshape(3, 3, 2, 128, 2, 128).transpose(5, 0, 1, 4, 2, 3)
    return np.ascontiguousarray(a.reshape(128, 36, 128)).astype(NP_BF16)


def _prep(inputs):
    x = np.asarray(inputs["x"], dtype=np.float32)

    fw, fb = {}, {}
    for n in ["up", "down", "p", "c1", "c2"]:
        g = np.asarray(inputs[f"g_{n}"], np.float32)
        v = np.asarray(inputs[f"v_{n}"], np.float32)
        m = np.asarray(inputs[f"m_{n}"], np.float32)
        b = np.asarray(inputs[f"b_{n}"], np.float32)
        w = np.asarray(inputs[f"w_{n}"], np.float32)
        s = g / np.sqrt(v + EPS)
        fw[n] = w * s[:, None, None, None]
        fb[n] = b - m * s

    sx = QCLIP / max(np.abs(x).max(), 1e-30)

    def wscale(w):
        m2 = np.abs(w).reshape(2, -1).max(axis=1)
        return QCLIP / np.maximum(m2, 1e-30)

    swup, swdn, swc1 = wscale(fw["up"]), wscale(fw["down"]), wscale(fw["c1"])
    qs = {
        "up": [float(1.0 / (sx * swup[t])) for t in range(2)],
        "dn": [float(1.0 / (sx * swdn[t])) for t in range(2)],
        "c1": [float(1.0 / (sx * swc1[t])) for t in range(2)],
    }

    wp = fw["p"]
    wa_stack = np.stack([
        np.stack([wp[:, :, ky, :].sum(-1) for ky in range(3)]),            # M
        np.stack([wp[:, :, ky, 1:].sum(-1) for ky in range(3)]),           # L (w=0)
        np.stack([wp[:, :, ky, :2].sum(-1) for ky in range(3)]),           # R (w=127)
    ])
    wb_stack = np.stack([
        np.stack([wp[:, :, :, kx].sum(-1) for kx in range(3)]),            # M
        np.stack([wp[:, :, 0, kx] for kx in range(3)]),                    # ky=0
        np.stack([wp[:, :, 2, kx] for kx in range(3)]),                    # ky=2
    ])
    consts = {
        "wup": _pack3_fp8(fw["up"], swup),
        "wdn": _pack3_fp8(fw["down"], swdn),
        "wc1": _pack1_fp8(fw["c1"], swc1),
        "wc2": _pack3(fw["c2"]),
        "wa": _pack_sep(wa_stack),
        "wb": _pack_sep(wb_stack),
    }
    bias_np = np.zeros((128, 8), np.float32)
    for k, arr in enumerate([fb["up"], fb["down"], fb["p"] + fb["c1"], fb["c2"]]):
        m2 = arr.reshape(2, 128)
        bias_np[:, 2 * k] = m2[0]
        bias_np[:, 2 * k + 1] = m2[1]
    consts["biases"] = bias_np

    def _bcast(row):
        return np.ascontiguousarray(
            np.broadcast_to(row.astype(np.float32)[None, :], (128, HS)))

    in_maps = []
    for core in range(N_CORES):
        b_i, half = core // 2, core % 2
        slab = np.zeros((256, HS, WPX), np.float32)
        if half == 0:
            slab[:, G:, 1:W + 1] = x[b_i][:, 0:HS - G, :]
            hv_row = (np.arange(HS) >= G)
            top_s, bot_s = G, None            # slab row of global row 0
        else:
            slab[:, :HS - G, 1:W + 1] = x[b_i][:, H - (HS - G):H, :]
            hv_row = (np.arange(HS) <= HS - G - 1)
            top_s, bot_s = None, HS - G - 1   # slab row of global row H-1
        x8 = np.ascontiguousarray(
            _q8(slab, sx).reshape(2, 128, HS, WPX).transpose(1, 0, 2, 3))
        pneg_row = np.where(hv_row, 0.0, NEG)
        htop_row = np.zeros(HS)
        if top_s is not None:
            htop_row[top_s] = -1.0
        hbot_row = np.zeros(HS)
        if bot_s is not None:
            hbot_row[bot_s] = -1.0
        in_maps.append({
            "x8": x8, "hv": _bcast(hv_row), "pnegb": _bcast(pneg_row),
            "htopneg": _bcast(htop_row), "hbotneg": _bcast(hbot_row), **consts})
    return in_maps, qs


def _run(inputs, trace=False):
    # Build a fresh Bass program per call: re-executing an already-loaded
    # NEFF on these cores intermittently trips NRT_EXEC_UNIT_UNRECOVERABLE,
    # while a fresh build+load is reliable (neuronxcc cache keeps it fast).
    in_maps, qs = _prep(inputs)
    nc = _build(qs)
    res = bass_utils.run_bass_kernel_spmd(
        nc, in_maps, core_ids=list(range(N_CORES)), trace=trace)
    out = np.empty((B, C, H, W), np.float32)
    for core in range(N_CORES):
        b_i, half = core // 2, core % 2
        r = np.asarray(res.results[core]["out"]).reshape(256, 64, W)
        out[b_i, :, half * 64:(half + 1) * 64, :] = r.astype(np.float32)
    return out, res


def kernel(**inputs) -> np.ndarray:
    out, _ = _run(inputs, trace=False)
    return out


# revision 17
# speedup vs baseline: 1.0071x; 1.0071x over previous
"""CenterPooling (CornerNet) Trainium2 kernel — 8 NeuronCores.

Sharding: 8 cores = 4 batches x 2 H-halves.  Each core gets a host-padded
input slab (3 halo rows each side, zero W-pad columns).

Key algebraic simplifications:
 - cummax(reverse) then cummax(forward) along an axis == global max along
   that axis, broadcast.  So the up branch only needs per-row maxes over W
   ([C, H]) and the down branch per-column maxes over H ([C, W]).
 - BN (eval mode) folds into conv weights/bias on the host; BN scale > 0 so
   max-reduction commutes with the affine+ReLU epilogue.
 - The merge conv's input is rank-structured: updown[c,h,w] = u[c,h] + d[c,w],
   so the 3x3 merge conv SEPARATES into tiny 1-D convs: an h-conv of u
   ([C, H] -> A(o,h), with 3 w-boundary classes of kx-summed weights) plus a
   w-conv of d ([C, W] -> B(o,w), with h-boundary corrections applied
   data-driven via per-row selector vectors).
 - Down-branch col-max needs a cross-half combine: pairwise AllReduce-max of
   a tiny [256, 128] tile.
 - H-pad semantics at the global top/bottom are handled data-driven (SPMD
   uniform program): a validity mask zeroes invalid u rows, and a per-row
   -1e30 bias on relu1 clamps out-of-range rows to the zero-pad value.

Precision: the up/down/c1 convs run as fp8e4 (TRN FP8_EXP4, max 240)
DoubleRow matmuls; the output conv block (c2) and the tiny separable merge
matmuls stay bf16 (fp8 there pushes rel_err past the 2e-2 gate; measured
~1e-2 with this split).  Accumulation is always fp32 PSUM.  Output is
stored bf16 and upcast on the host (~0.1% extra L2, halves store traffic).

Schedule notes (v2):
 - Input DMA descriptors are issued from BOTH hardware DGE queues (sync +
   scalar) so descriptor generation (~0.7us each) does not serialize the
   startup: weights on sync, the x8 slab on scalar.
 - ufin/umask are produced incrementally per up-conv block, so the A-conv
   matmuls can start the moment the last up block drains; the B-conv MMs
   run right after A, keeping the PE busy while the A/B epilogue chains
   (psum copies + afull/afdl/afdr) run on scalar/vector.
 - relu1 assembly is batched per 4-row block with broadcast APs (one STT +
   one add + one ACT per block-cot instead of per-row ops), shortening the
   exposed chain at the conv tail.
 - the last c2 block is split 2+2 rows to shorten the final ACT+store tail.
"""

import sys

sys.path.insert(0, "/opt/trn_rl_repo")

import numpy as np
import ml_dtypes

import concourse.bacc as bacc
import concourse.tile as tile
import concourse.bass as bass
from concourse import mybir, bass_utils

BF16 = mybir.dt.bfloat16
FP8 = mybir.dt.float8e4
F32 = mybir.dt.float32
NP_BF16 = ml_dtypes.bfloat16
NP_FP8 = ml_dtypes.float8_e4m3  # IEEE e4m3, max 240 == TRN FP8_EXP4

N_CORES = 8
B, CIN, C, H, W = 4, 256, 256, 128, 128
G = 3            # halo rows on each side of the 64 owned rows
HS = 64 + 2 * G  # 70 slab rows
WP = W + 2       # 130 (zero-pad col on each side) — bf16 r1 slab
WPX = 144        # fp8 x slab width: 16B-aligned row and plane strides
EPS = 1e-5
NEG = -1e30
QCLIP = 224.0    # fp8 absmax target (headroom under the 240 max)

RELU = mybir.ActivationFunctionType.Relu
AX_X = mybir.AxisListType.X
ALU = mybir.AluOpType
DROW = mybir.MatmulPerfMode.DoubleRow


def _mm_group(nc, ps_ap, mms, perf_mode=None):
    n = len(mms)
    for k, (lhsT, rhs) in enumerate(mms):
        nc.tensor.matmul(ps_ap, lhsT, rhs, start=(k == 0), stop=(k == n - 1),
                         perf_mode=perf_mode)


def _conv3_mms8(wtile, x8, s, nr, cot):
    """The 9 (ky,kx) DoubleRow matmuls of a 3x3 conv: output rows s..s+nr-1.

    wtile is [128ci, 2cit, 2cot, 9j, 128co]; the lhsT AP [128, 2, 128] takes
    the cit planes at stride 2*9*128 (16B-aligned as DoubleRow requires)."""
    mms = []
    for ky in range(3):
        for kx in range(3):
            mms.append((wtile[:, :, cot, ky * 3 + kx, :],
                        x8[:, :, s + ky - 1:s + ky - 1 + nr, kx:kx + W]))
    return mms


def _conv3_mms(wtile, src, s, nr, cot):
    """The 18 (ci,ky,kx) bf16 matmuls of a 3x3 conv block."""
    mms = []
    for cit in range(2):
        for ky in range(3):
            for kx in range(3):
                j = ((ky * 3 + kx) * 2 + cit) * 2 + cot
                mms.append((wtile[:, j, :], src[cit][:, s + ky - 1:s + ky - 1 + nr, kx:kx + W]))
    return mms


def _build(qs):
    nc = bacc.Bacc("TRN2", target_bir_lowering=False, debug=False,
                   num_devices=N_CORES)

    x8_d = nc.dram_tensor("x8", [128, 2, HS, WPX], FP8, kind="ExternalInput")
    # (cit, cot) are OUTERMOST in dram so each per-(cit,cot) DMA is fully
    # contiguous per partition; the first down MM then only gates on the
    # cot=0 halves.
    wup_d = nc.dram_tensor("wup", [2, 2, 128, 9, 128], FP8, kind="ExternalInput")
    wdn_d = nc.dram_tensor("wdn", [2, 2, 128, 9, 128], FP8, kind="ExternalInput")
    wc1_d = nc.dram_tensor("wc1", [2, 128, 2, 128], FP8, kind="ExternalInput")
    wc2_d = nc.dram_tensor("wc2", [128, 36, 128], BF16, kind="ExternalInput")
    wa_d = nc.dram_tensor("wa", [128, 36, 128], BF16, kind="ExternalInput")
    wb_d = nc.dram_tensor("wb", [128, 36, 128], BF16, kind="ExternalInput")
    bias_d = nc.dram_tensor("biases", [128, 8], F32, kind="ExternalInput")
    hv_d = nc.dram_tensor("hv", [128, HS], F32, kind="ExternalInput")
    pnegb_d = nc.dram_tensor("pnegb", [128, HS], F32, kind="ExternalInput")
    htop_d = nc.dram_tensor("htopneg", [128, HS], F32, kind="ExternalInput")
    hbot_d = nc.dram_tensor("hbotneg", [128, HS], F32, kind="ExternalInput")
    out_d = nc.dram_tensor("out", [2, 128, 64, W], BF16, kind="ExternalOutput")

    with tile.TileContext(nc) as tc:
        with tc.tile_pool(name="const", bufs=1) as constp, \
             tc.tile_pool(name="acts", bufs=1) as actp, \
             tc.tile_pool(name="psum", bufs=6, space="PSUM") as psp, \
             tc.tile_pool(name="ostage", bufs=6) as osp, \
             tc.tile_pool(name="dram", bufs=1, space="DRAM") as dramp:

            # --- input DMA: weights on the sync queue, x8 on the scalar
            # queue (both are hardware DGE engines) so descriptor
            # generation overlaps and the first conv can start sooner.
            wdn = constp.tile([128, 2, 2, 9, 128], FP8)
            for cot in range(2):
                for cit in range(2):
                    nc.sync.dma_start(wdn[:, cit, cot, :, :],
                                      wdn_d.ap()[cit, cot, :, :, :])

            x8 = actp.tile([128, 2, HS, WPX], FP8, name="x8")
            # first 7 rows of both cits land first (feeds the first down and
            # up blocks), then 16-row chunks
            row_chunks = [(0, 7), (7, 23), (23, 39), (39, 55), (55, HS)]
            for r0, r1_ in row_chunks:
                for cit in range(2):
                    nc.scalar.dma_start(x8[:, cit, r0:r1_, :], x8_d.ap()[:, cit, r0:r1_, :])

            wup = constp.tile([128, 2, 2, 9, 128], FP8)
            for cot in range(2):
                for cit in range(2):
                    nc.sync.dma_start(wup[:, cit, cot, :, :],
                                      wup_d.ap()[cit, cot, :, :, :])
            wc1 = constp.tile([128, 2, 2, 128], FP8)
            for cit in range(2):
                nc.sync.dma_start(wc1[:, cit, :, :], wc1_d.ap()[cit, :, :, :])
            biases = constp.tile([128, 8], F32)
            nc.sync.dma_start(biases[:, :], bias_d.ap())
            hv = constp.tile([128, HS], F32)
            nc.sync.dma_start(hv[:, :], hv_d.ap())
            wa = constp.tile([128, 36, 128], BF16)
            nc.sync.dma_start(wa[:, :, :], wa_d.ap())
            wb = constp.tile([128, 36, 128], BF16)
            nc.sync.dma_start(wb[:, :, :], wb_d.ap())
            pnegb = constp.tile([128, HS], F32)
            nc.sync.dma_start(pnegb[:, :], pnegb_d.ap())
            htopneg = constp.tile([128, HS], F32)
            nc.sync.dma_start(htopneg[:, :], htop_d.ap())
            hbotneg = constp.tile([128, HS], F32)
            nc.sync.dma_start(hbotneg[:, :], hbot_d.ap())
            wc2 = constp.tile([128, 36, 128], BF16)
            nc.sync.dma_start(wc2[:, :, :], wc2_d.ap())

            r1 = []
            for cit in range(2):
                t2 = actp.tile([128, HS, WP], BF16, name=f"r1{cit}")
                nc.vector.memset(t2[:, :, 0], 0.0)
                nc.vector.memset(t2[:, :, WP - 1], 0.0)
                r1.append(t2)

            uraw, ufin, umask, dacc, dmax, dfin = [], [], [], [], [], []
            for cot in range(2):
                uraw.append(actp.tile([128, HS], F32, name=f"uraw{cot}"))
                ufin.append(actp.tile([128, HS], F32, name=f"ufin{cot}"))
                umask.append(actp.tile([128, HS], BF16, name=f"umask{cot}"))
                t = actp.tile([128, W], F32, name=f"dacc{cot}")
                nc.vector.memset(t[:, :], -3e38)
                dacc.append(t)
                dmax.append(actp.tile([128, W], F32, name=f"dmax{cot}"))
                dfin.append(actp.tile([128, W], F32, name=f"dfin{cot}"))

            # ---- down branch: fp8 conv over the 64 owned rows, col-max over H ----
            for i in range(16):
                s = G + 4 * i
                for cot in range(2):
                    ps = psp.tile([128, 4, 128], F32, tag="ps", name="ps_dn", bufs=4)
                    _mm_group(nc, ps[:, :, :], _conv3_mms8(wdn, x8, s, 4, cot),
                              perf_mode=DROW)
                    for rr in range(4):
                        nc.vector.tensor_max(dacc[cot][:, :], dacc[cot][:, :], ps[:, rr, :])

            # pairwise (same-batch) AllReduce-max to get the global col-max
            # (values carry the sx*swdn quant scale; both group members match)
            cc_in = dramp.tile([256, W], F32)
            cc_out = dramp.tile([256, W], F32)
            for cot in range(2):
                nc.sync.dma_start(cc_in[cot * 128:(cot + 1) * 128, :], dacc[cot][:, :])
            nc.gpsimd.collective_compute(
                "AllReduce", ALU.max,
                replica_groups=[[0, 1], [2, 3], [4, 5], [6, 7]],
                ins=[cc_in.opt()], outs=[cc_out.opt()])
            for cot in range(2):
                nc.sync.dma_start(dmax[cot][:, :], cc_out[cot * 128:(cot + 1) * 128, :])

            # ---- up branch: fp8 conv over rows [1, 69), row-max over W ----
            # ufin/umask are produced per block; the merge pieces (dfin/dpad,
            # the B conv, the first half of the A conv) and the first relu1
            # blocks are woven INTO the up-conv stream so no dependency chain
            # is exposed when the up conv drains.
            NA = 64 + 2        # A-conv output rows 2..67
            NA1 = 40           # half 1: rows 2..41  (needs umask 1..42)
            NA2 = NA - NA1     # half 2: rows 42..67 (needs umask 41..68)
            dpad = [None, None]
            asb = [[None, None, None], [None, None, None]]
            bt = [[None, None, None], [None, None, None]]
            afull, afdl, afdr = [], [], []
            for cot in range(2):
                afull.append(actp.tile([128, HS], F32, name=f"afull{cot}"))
                afdl.append(actp.tile([128, HS], F32, name=f"afdl{cot}"))
                afdr.append(actp.tile([128, HS], F32, name=f"afdr{cot}"))

            def emit_dfin_dpad():
                for cot in range(2):
                    nc.scalar.activation(dfin[cot][:, :], dmax[cot][:, :], RELU,
                                         bias=biases[:, 2 + cot:3 + cot],
                                         scale=qs["dn"][cot])
                    t = actp.tile([128, WP], BF16, name=f"dpad{cot}")
                    nc.vector.memset(t[:, :], 0.0)
                    nc.vector.tensor_copy(t[:, 1:W + 1], dfin[cot][:, :])
                    dpad[cot] = t

            def emit_a_half(r0, na):
                # A_cls(o,h): 1-D h-conv of umask with kx-summed merge
                # weights; cls 0=M (interior w), 1=L (w=0), 2=R (w=127).
                # Output rows r0..r0+na-1 into asb columns [r0-2, r0-2+na).
                for cls in range(3):
                    for cot in range(2):
                        psa_t = psp.tile([128, 4, 128], F32, tag="ps2", name="ps_a", bufs=2)
                        mms = []
                        for cit in range(2):
                            for ky in range(3):
                                j = ((cls * 3 + ky) * 2 + cit) * 2 + cot
                                mms.append((wa[:, j, :],
                                            umask[cit][:, r0 - 1 + ky:r0 - 1 + ky + na]))
                        _mm_group(nc, psa_t[:, 0, 0:na], mms)
                        if asb[cot][cls] is None:
                            asb[cot][cls] = actp.tile([128, NA], F32, name=f"asb{cls}{cot}")
                        nc.scalar.copy(asb[cot][cls][:, r0 - 2:r0 - 2 + na],
                                       psa_t[:, 0, 0:na])
                # afull = A_M + bias_pc1 + pneg (per relu1 row);
                # afdL/afdR = A_L - A_M / A_R - A_M (w-edge fixups, pre-ReLU).
                for cot in range(2):
                    nc.vector.scalar_tensor_tensor(
                        afull[cot][:, r0:r0 + na], asb[cot][0][:, r0 - 2:r0 - 2 + na],
                        biases[:, 4 + cot:5 + cot], pnegb[:, r0:r0 + na],
                        op0=ALU.add, op1=ALU.add)
                    nc.vector.tensor_sub(afdl[cot][:, r0:r0 + na],
                                         asb[cot][1][:, r0 - 2:r0 - 2 + na],
                                         asb[cot][0][:, r0 - 2:r0 - 2 + na])
                    nc.vector.tensor_sub(afdr[cot][:, r0:r0 + na],
                                         asb[cot][2][:, r0 - 2:r0 - 2 + na],
                                         asb[cot][0][:, r0 - 2:r0 - 2 + na])

            def emit_b():
                # B_var(o,w): 1-D w-conv of dpad with ky-summed merge
                # weights; var 0=M (all ky), 1=ky0 only, 2=ky2 only.
                for var in range(3):
                    for cot in range(2):
                        psb_t = psp.tile([128, 4, 128], F32, tag="ps2", name="ps_b", bufs=2)
                        mms = []
                        for cit in range(2):
                            for kx in range(3):
                                j = ((var * 3 + kx) * 2 + cit) * 2 + cot
                                mms.append((wb[:, j, :], dpad[cit][:, kx:kx + W]))
                        _mm_group(nc, psb_t[:, 0, :], mms)
                        t = actp.tile([128, 128], F32, name=f"bt{var}{cot}")
                        nc.vector.tensor_copy(t[:, :], psb_t[:, 0, :])
                        bt[cot][var] = t

            def btaf_prep(s, nr, cot):
                # btaf[:, r, w] = bt_M[:, w] + afull[:, s+r] + edge fixups —
                # precomputed on vector OFF the c1->relu1 critical chain.
                t = osp.tile([128, 4, 128], F32, tag="btaf", name="btaf", bufs=6)
                bt_b = bt[cot][0][:, :].unsqueeze(1).broadcast_to([128, nr, 128])
                af_b = afull[cot][:, s:s + nr].unsqueeze(2).broadcast_to([128, nr, 128])
                nc.vector.tensor_add(t[:, 0:nr, :], bt_b, af_b)
                nc.vector.tensor_add(t[:, 0:nr, 0], t[:, 0:nr, 0], afdl[cot][:, s:s + nr])
                nc.vector.tensor_add(t[:, 0:nr, W - 1], t[:, 0:nr, W - 1], afdr[cot][:, s:s + nr])
                for r in range(nr):
                    sr = s + r
                    if sr == G:
                        nc.vector.scalar_tensor_tensor(
                            t[:, r, :], bt[cot][1][:, :], htopneg[:, sr:sr + 1],
                            t[:, r, :], op0=ALU.mult, op1=ALU.add)
                    if sr == HS - G - 1:
                        nc.vector.scalar_tensor_tensor(
                            t[:, r, :], bt[cot][2][:, :], hbotneg[:, sr:sr + 1],
                            t[:, r, :], op0=ALU.mult, op1=ALU.add)
                return t

            btafs = {}

            def emit_relu1_prep(s, nr):
                # emitted one step AHEAD of the mm stage so the vector queue
                # has the btaf ready when the c1 MM issues
                btafs[s] = [btaf_prep(s, nr, cot) for cot in range(2)]

            def emit_relu1_mms(s, nr, tag="ps", bufs=4):
                # relu1 = relu(c1(x) + A + B + bias): c1 MM -> STT -> ACT
                for cot in range(2):
                    btaf = btafs[s][cot]
                    ps = psp.tile([128, 4, 128], F32, tag=tag, name="ps_p", bufs=bufs)
                    nc.tensor.matmul(ps[:, 0:nr, :], wc1[:, :, cot, :],
                                     x8[:, :, s:s + nr, 1:W + 1],
                                     start=True, stop=True, perf_mode=DROW)
                    nc.vector.scalar_tensor_tensor(
                        ps[:, 0:nr, :], ps[:, 0:nr, :], qs["c1"][cot],
                        btaf[:, 0:nr, :], op0=ALU.mult, op1=ALU.add)
                    nc.scalar.activation(r1[cot][:, s:s + nr, 1:W + 1], ps[:, 0:nr, :],
                                         RELU, bias=0.0, scale=1.0)

            # up-conv blocks b=1..17 with merge work woven in
            for b in range(1, 18):
                s = 1 + 4 * (b - 1)
                for cot in range(2):
                    ps = psp.tile([128, 4, 128], F32, tag="ps", name="ps_up", bufs=4)
                    _mm_group(nc, ps[:, :, :], _conv3_mms8(wup, x8, s, 4, cot),
                              perf_mode=DROW)
                    nc.vector.reduce_max(uraw[cot][:, s:s + 4], ps[:, :, :], axis=AX_X)
                    nc.scalar.activation(ufin[cot][:, s:s + 4], uraw[cot][:, s:s + 4],
                                         RELU, bias=biases[:, cot:cot + 1],
                                         scale=qs["up"][cot])
                    nc.vector.tensor_mul(umask[cot][:, s:s + 4], ufin[cot][:, s:s + 4],
                                         hv[:, s:s + 4])
                if b == 10:
                    emit_dfin_dpad()       # collective done long before
                elif b == 11:
                    emit_a_half(2, NA1)    # umask rows 1..42 available
                    emit_b()
                elif b in (12, 14, 16):
                    # a few early relu1 blocks, spread so the vector queue
                    # keeps pace with the up conv; preps go one up-block
                    # ahead of the matmul stage, and the woven c1 MMs use a
                    # dedicated psum tag so the up-conv bank rotation never
                    # couples to the relu1 STT chain
                    emit_relu1_prep(2 + 4 * ((b - 12) // 2), 4)
                elif b in (13, 15, 17):
                    emit_relu1_mms(2 + 4 * ((b - 13) // 2), 4, tag="pse", bufs=2)
                    if b == 17:
                        emit_relu1_prep(14, 4)

            # ---- c2 for the rows already assembled, then A half 2, then the
            # remaining relu1 and c2 blocks interleaved so the PE always has
            # a c2 block queued while relu1 chains run ----
            def emit_c2_block(s, nr):
                for cot in range(2):
                    ps = psp.tile([128, 4, 128], F32, tag="ps2", name="ps_c2", bufs=2)
                    _mm_group(nc, ps[:, 0:nr, :], _conv3_mms(wc2, r1, s, nr, cot))
                    ot = osp.tile([128, 4, 128], BF16, name="ot")
                    nc.scalar.activation(ot[:, 0:nr, :], ps[:, 0:nr, :], RELU,
                                         bias=biases[:, 6 + cot:7 + cot], scale=1.0)
                    if s >= G + 56:
                        # split the tail stores across rings so the last
                        # store's serial latency is halved
                        for r in range(0, nr, 2):
                            nc.sync.dma_start(out_d.ap()[cot, :, s - G + r:s - G + r + 2, :],
                                              ot[:, r:r + 2, :])
                    else:
                        nc.sync.dma_start(out_d.ap()[cot, :, s - G:s - G + nr, :], ot[:, 0:nr, :])

            emit_c2_block(G, 4)     # needs only relu1 rows 2..7 (done mid-up)
            emit_a_half(2 + NA1, NA2)

            r1_rest = [(14 + 4 * i, 4) for i in range(13)] + [(66, 2)]
            c2rest = [(G + 4 * (i + 1), 4) for i in range(14)] + [(G + 60, 2), (G + 62, 2)]
            for k in range(len(c2rest)):
                if k < len(r1_rest):
                    emit_relu1_mms(*r1_rest[k])
                if k + 1 < len(r1_rest):
                    emit_relu1_prep(*r1_rest[k + 1])
                emit_c2_block(*c2rest[k])

    nc.compile()
    return nc


def _pack3(w):
    # [256o, 256i, 3, 3] -> [128ci, j, 128co], j = ((ky*3+kx)*2+cit)*2+cot
    a = w.reshape(2, 128, 2, 128, 3, 3).transpose(3, 4, 5, 2, 0, 1)
    return np.ascontiguousarray(a.reshape(128, 36, 128)).astype(NP_BF16)


def _q8(a, s):
    return np.clip(a * s, -240.0, 240.0).astype(NP_FP8)


def _pack3_fp8(w, s_cot):
    # [256o, 256i, 3, 3] -> [2cit, 2cot, 128ci, j=ky*3+kx, 128co] fp8,
    # scaled per output-channel tile (cot)
    ws = w * np.repeat(s_cot, 128)[:, None, None, None]
    a = ws.reshape(2, 128, 2, 128, 3, 3).transpose(2, 0, 3, 4, 5, 1)
    # dims now [cit, cot, ci, ky, kx, co]
    return np.ascontiguousarray(_q8(a.reshape(2, 2, 128, 9, 128), 1.0))


def _pack1_fp8(w, s_cot):
    # [256o, 256i, 1, 1] -> [2cit, 128ci, cot, 128co] fp8
    ws = w[:, :, 0, 0] * np.repeat(s_cot, 128)[:, None]
    a = ws.reshape(2, 128, 2, 128).transpose(2, 3, 0, 1)
    return np.ascontiguousarray(_q8(a, 1.0))


def _pack_sep(wk3):
    # packs a [3var/cls, 3k, 256, 256] stack into [128ci, j, 128co],
    # j = ((v*3+k)*2+cit)*2+cot
    a = wk3.reshape(3, 3, 2, 128, 2, 128).transpose(5, 0, 1, 4, 2, 3)
    return np.ascontiguousarray(a.reshape(128, 36, 128)).astype(NP_BF16)


def _prep(inputs):
    x = np.asarray(inputs["x"], dtype=np.float32)

    fw, fb = {}, {}
    for n in ["up", "down", "p", "c1", "c2"]:
        g = np.asarray(inputs[f"g_{n}"], np.float32)
        v = np.asarray(inputs[f"v_{n}"], np.float32)
        m = np.asarray(inputs[f"m_{n}"], np.float32)
        b = np.asarray(inputs[f"b_{n}"], np.float32)
        w = np.asarray(inputs[f"w_{n}"], np.float32)
        s = g / np.sqrt(v + EPS)
        fw[n] = w * s[:, None, None, None]
        fb[n] = b - m * s

    sx = QCLIP / max(np.abs(x).max(), 1e-30)

    def wscale(w):
        m2 = np.abs(w).reshape(2, -1).max(axis=1)
        return QCLIP / np.maximum(m2, 1e-30)

    swup, swdn, swc1 = wscale(fw["up"]), wscale(fw["down"]), wscale(fw["c1"])
    qs = {
        "up": [float(1.0 / (sx * swup[t])) for t in range(2)],
        "dn": [float(1.0 / (sx * swdn[t])) for t in range(2)],
        "c1": [float(1.0 / (sx * swc1[t])) for t in range(2)],
    }

    wp = fw["p"]
    wa_stack = np.stack([
        np.stack([wp[:, :, ky, :].sum(-1) for ky in range(3)]),            # M
        np.stack([wp[:, :, ky, 1:].sum(-1) for ky in range(3)]),           # L (w=0)
        np.stack([wp[:, :, ky, :2].sum(-1) for ky in range(3)]),           # R (w=127)
    ])
    wb_stack = np.stack([
        np.stack([wp[:, :, :, kx].sum(-1) for kx in range(3)]),            # M
        np.stack([wp[:, :, 0, kx] for kx in range(3)]),                    # ky=0
        np.stack([wp[:, :, 2, kx] for kx in range(3)]),                    # ky=2
    ])
    consts = {
        "wup": _pack3_fp8(fw["up"], swup),
        "wdn": _pack3_fp8(fw["down"], swdn),
        "wc1": _pack1_fp8(fw["c1"], swc1),
        "wc2": _pack3(fw["c2"]),
        "wa": _pack_sep(wa_stack),
        "wb": _pack_sep(wb_stack),
    }
    bias_np = np.zeros((128, 8), np.float32)
    for k, arr in enumerate([fb["up"], fb["down"], fb["p"] + fb["c1"], fb["c2"]]):
        m2 = arr.reshape(2, 128)
        bias_np[:, 2 * k] = m2[0]
        bias_np[:, 2 * k + 1] = m2[1]
    consts["biases"] = bias_np

    def _bcast(row):
        return np.ascontiguousarray(
            np.broadcast_to(row.astype(np.float32)[None, :], (128, HS)))

    in_maps = []
    for core in range(N_CORES):
        b_i, half = core // 2, core % 2
        slab = np.zeros((256, HS, WPX), np.float32)
        if half == 0:
            slab[:, G:, 1:W + 1] = x[b_i][:, 0:HS - G, :]
            hv_row = (np.arange(HS) >= G)
            top_s, bot_s = G, None            # slab row of global row 0
        else:
            slab[:, :HS - G, 1:W + 1] = x[b_i][:, H - (HS - G):H, :]
            hv_row = (np.arange(HS) <= HS - G - 1)
            top_s, bot_s = None, HS - G - 1   # slab row of global row H-1
        x8 = np.ascontiguousarray(
            _q8(slab, sx).reshape(2, 128, HS, WPX).transpose(1, 0, 2, 3))
        pneg_row = np.where(hv_row, 0.0, NEG)
        htop_row = np.zeros(HS)
        if top_s is not None:
            htop_row[top_s] = -1.0
        hbot_row = np.zeros(HS)
        if bot_s is not None:
            hbot_row[bot_s] = -1.0
        in_maps.append({
            "x8": x8, "hv": _bcast(hv_row), "pnegb": _bcast(pneg_row),
            "htopneg": _bcast(htop_row), "hbotneg": _bcast(hbot_row), **consts})
    return in_maps, qs


def _run(inputs, trace=False):
    # Build a fresh Bass program per call: re-executing an already-loaded
    # NEFF on these cores intermittently trips NRT_EXEC_UNIT_UNRECOVERABLE,
    # while a fresh build+load is reliable (neuronxcc cache keeps it fast).
    in_maps, qs = _prep(inputs)
    nc = _build(qs)
    res = bass_utils.run_bass_kernel_spmd(
        nc, in_maps, core_ids=list(range(N_CORES)), trace=trace)
    out = np.empty((B, C, H, W), np.float32)
    for core in range(N_CORES):
        b_i, half = core // 2, core % 2
        r = np.asarray(res.results[core]["out"]).reshape(256, 64, W)
        out[b_i, :, half * 64:(half + 1) * 64, :] = r.astype(np.float32)
    return out, res


def kernel(**inputs) -> np.ndarray:
    out, _ = _run(inputs, trace=False)
    return out
